# revision 1
# baseline (speedup 1.0000x reference)
"""Trainium2 Bass kernel for nn_AttentionWeight (GAT edge softmax).

out[e,h] = softmax_over_dst_segments(relu(el[src]+er[dst]+ee[etype]))

Math used on device:
  exp(relu(x)) = max(exp(x), 1)  and  exp(x) = exp(el+ee)*exp(er)
  y := exp(relu(x)) - 1 = max(exp(el+ee)*exp(er) - 1, 0)
  segment_sum(exp(relu(x))) = sum(y) + deg   (padding slots give y = 0)
  out = (y + 1) * reciprocal(segment_sum)    (softmax is shift-invariant, the
                                              reference's max-subtraction is
                                              only for numerical range; values
                                              here are O(1) so it is not needed)

Distribution (8 NeuronCores):
  Launch A: node-sharded projections. Core s owns nodes [12500s, 12500(s+1)):
    el/er = feat @ (W_fc contracted with attn_l/attn_r) -> exp'd; the tiny
    edge-type table ee' = exp(contract(edge_emb@W_e, attn_e)); and the
    combined gather table el8[(n,t)] = el'[n]*ee'[t] for its node shard.
  Host: concatenates per-core el8 shards (pure relabeling, no arithmetic).
  Launch B: edge/dst-sharded softmax. Core c owns dst in [12500c, 12500(c+1)).
    Edges are dst-sorted and padded into [128 nodes x D_g] groups (nodes
    degree-sorted so groups are tight, ~1.5%% padding). One [128,1]-indexed
    indirect DMA gathers one slot column (128 rows of 32B) from el8; walrus
    miscompiles multi-index offset APs, so one instruction per column is the
    only correct form, and its ~1us SWDGE fixed cost on the Pool engine is
    the kernel's dominant term. Per group: multiply by broadcast er', the
    max(m-1,0) trick, a strided X-reduce for segment sums, reciprocal, and
    (y+1)*r, then store the padded slots.
  Host: scatters padded slots back to original edge order (indexing only).

All floating-point arithmetic happens on device; the host only shards,
permutes, concatenates and builds integer index/count arrays.
"""

import sys

sys.path.insert(0, "/opt/trn_rl_repo")

import numpy as np

import concourse.bass as bass
import concourse.bacc as bacc
import concourse.mybir as mybir
import concourse.tile as tile
from concourse.bass_utils import run_bass_kernel_spmd

# problem constants (hardcoded per harness contract)
N = 100000
E = 3200000
IN = 256
H = 8
O = 64
F = 64
T = 8
NCORES = 8
P = 128

NS = N // NCORES            # 12500 nodes per shard
NSP = 12544                 # padded to 128*98
G = NSP // P                # 98 groups of 128 nodes
ELFULL_ROWS = 128 * 785     # 100480: 8*12544=100352 real rows + pad
SENTINEL = 100352           # zero row in el_full -> el8 row SENTINEL*8 is 0
EL8_ROWS = ELFULL_ROWS * 8

FP = mybir.dt.float32
I32 = mybir.dt.int32

_timings = {}


# ---------------------------------------------------------------------------
# Launch A: projections
# ---------------------------------------------------------------------------

def _build_launch_a():
    nc = bacc.Bacc("TRN2", target_bir_lowering=False, debug=False,
                   num_devices=NCORES)
    featT = nc.dram_tensor("featT", [IN, NSP], FP, kind="ExternalInput")
    w_fc = nc.dram_tensor("w_fc", [IN, H * O], FP, kind="ExternalInput")
    attn_lr = nc.dram_tensor("attn_lr", [P, 2 * H * O], FP, kind="ExternalInput")
    edge_embT = nc.dram_tensor("edge_embT", [F, T], FP, kind="ExternalInput")
    w_e = nc.dram_tensor("w_e", [F, H * F], FP, kind="ExternalInput")
    attn_e = nc.dram_tensor("attn_e", [T, H * F], FP, kind="ExternalInput")
    erp = nc.dram_tensor("erp", [NSP, H], FP, kind="ExternalOutput")
    eep = nc.dram_tensor("eep", [T, H], FP, kind="ExternalOutput")
    el8s = nc.dram_tensor("el8s", [NSP * T, H], FP, kind="ExternalOutput")

    with tile.TileContext(nc) as tc:
        with (
            tc.tile_pool(name="sb", bufs=1) as sb,
            tc.tile_pool(name="mm", bufs=2) as mm,
            tc.tile_pool(name="ps", bufs=2, space="PSUM") as ps,
        ):
            # --- wl/wr: contract W_fc[i, h*O+o] with attn_l/r[h, o] -> [i, 2H]
            wfc_t = [sb.tile([P, H * O], FP, tag=f"wfc{c}", name=f"wfc{c}") for c in range(2)]
            for c in range(2):
                nc.sync.dma_start(wfc_t[c][:], w_fc[c * P:(c + 1) * P, :])
            alr_t = sb.tile([P, 2 * H * O], FP)
            nc.sync.dma_start(alr_t[:], attn_lr[:])
            wlr = [sb.tile([P, 2 * H], FP, tag=f"wlr{c}", name=f"wlr{c}") for c in range(2)]
            for c in range(2):
                for half in range(2):  # 0: attn_l, 1: attn_r
                    tmp = mm.tile([P, H * O], FP, tag="wtmp")
                    nc.vector.tensor_tensor(
                        tmp[:], wfc_t[c][:],
                        alr_t[:, half * H * O:(half + 1) * H * O],
                        mybir.AluOpType.mult)
                    nc.vector.tensor_reduce(
                        wlr[c][:, half * H:(half + 1) * H],
                        tmp[:].rearrange("p (h o) -> p h o", h=H),
                        mybir.AxisListType.X, mybir.AluOpType.add)

            # --- ee table: (edge_emb @ W_e) [T, H*F] contract attn_e -> [T, H]
            embT_t = sb.tile([F, T], FP)
            nc.sync.dma_start(embT_t[:], edge_embT[:])
            we_t = sb.tile([F, H * F], FP)
            nc.sync.dma_start(we_t[:], w_e[:])
            ae_t = sb.tile([T, H * F], FP)
            nc.sync.dma_start(ae_t[:], attn_e[:])
            proj_ps = ps.tile([T, H * F], FP)
            nc.tensor.matmul(proj_ps[:], lhsT=embT_t[:], rhs=we_t[:],
                             start=True, stop=True)
            proj_sb = sb.tile([T, H * F], FP)
            nc.vector.tensor_tensor(
                proj_sb[:], proj_ps[:], ae_t[:],
                mybir.AluOpType.mult)
            ee_sb = sb.tile([T, H], FP)
            nc.vector.tensor_reduce(
                ee_sb[:], proj_sb[:].rearrange("t (h f) -> t h f", h=H),
                mybir.AxisListType.X, mybir.AluOpType.add)
            eep_sb = sb.tile([T, H], FP)
            nc.scalar.activation(eep_sb[:], ee_sb[:],
                                 mybir.ActivationFunctionType.Exp)
            nc.sync.dma_start(eep[:], eep_sb[:])

            # --- el/er for the shard: node ln = p*G + tt handled by
            #     (tile tt, psum partition p)
            ftT = [sb.tile([P, NSP], FP, tag=f"ft{c}", name=f"ft{c}") for c in range(2)]
            for c in range(2):
                nc.sync.dma_start(ftT[c][:], featT[c * P:(c + 1) * P, :])
            elr = sb.tile([P, G, 2 * H], FP)
            # batch 32 node-tiles per single-bank PSUM tile ([128, 512] f32);
            # accumulation stays strictly sequential per 16-col slice (the
            # HW-verified pattern) -- only the exp drain is batched per bank.
            SLICES = 32
            tt = 0
            while tt < G:
                nsl = min(SLICES, G - tt)
                bank = ps.tile([P, SLICES * 2 * H], FP, tag="bank")
                for j in range(nsl):
                    sl = bank[:, j * 2 * H:(j + 1) * 2 * H]
                    for c in range(2):
                        lhsT = ftT[c][:].rearrange("i (p t) -> i t p", p=P)[:, tt + j, :]
                        nc.tensor.matmul(sl, lhsT=lhsT, rhs=wlr[c][:],
                                         start=(c == 0), stop=(c == 1))
                nc.scalar.activation(
                    elr[:, tt:tt + nsl, :],
                    bank[:, :nsl * 2 * H].rearrange("p (t h) -> p t h", h=2 * H),
                    mybir.ActivationFunctionType.Exp)
                tt += nsl
            # write out: partition p holds nodes [G*p, G*(p+1))
            nc.sync.dma_start(
                erp[:].rearrange("(p t) h -> p t h", p=P), elr[:, :, H:2 * H])
            # el8 shard: row (ln*T + t) = el'[ln] * ee'[t]
            eeb = sb.tile([P, T * H], FP)
            nc.sync.dma_start(
                eeb[:],
                eep[:].rearrange("t h -> (t h)").unsqueeze(0)
                .to_broadcast([P, T * H]))
            blk = sb.tile([P, G, T, H], FP)
            nc.vector.tensor_tensor(
                blk[:],
                elr[:, :, 0:H].unsqueeze(2).to_broadcast([P, G, T, H]),
                eeb[:].rearrange("p (t h) -> p t h", t=T).unsqueeze(1)
                .to_broadcast([P, G, T, H]),
                mybir.AluOpType.mult)
            nc.sync.dma_start(
                el8s[:].rearrange("(p g t) h -> p g t h", p=P, t=T), blk[:])

    nc.compile()
    return nc


# ---------------------------------------------------------------------------
# Launch B: edge softmax
# ---------------------------------------------------------------------------

def _build_launch_b(gds, ktot):
    """gds: per-group slot width D_g (len G); ktot = sum(gds)."""
    nc = bacc.Bacc("TRN2", target_bir_lowering=False, debug=False,
                   num_devices=NCORES)
    el8 = nc.dram_tensor("el8", [EL8_ROWS, H], FP, kind="ExternalInput")
    er_grid = nc.dram_tensor("er_grid", [P, G * H], FP, kind="ExternalInput")
    deg = nc.dram_tensor("deg", [P, G], FP, kind="ExternalInput")
    idx = nc.dram_tensor("idx", [P, ktot], I32, kind="ExternalInput")
    out = nc.dram_tensor("out", [P, ktot * H], FP, kind="ExternalOutput")

    with tile.TileContext(nc) as tc:
        # gather + softmax chain, one group of 128 dst nodes at a time
        with (
            tc.tile_pool(name="cst", bufs=1) as cst,
            tc.tile_pool(name="gp", bufs=3) as gp,
            tc.tile_pool(name="yp", bufs=3) as yp,
            tc.tile_pool(name="ip", bufs=3) as ip,
            tc.tile_pool(name="sp", bufs=3) as sp,
        ):
            er_sb = cst.tile([P, G, H], FP)
            nc.sync.dma_start(er_sb[:],
                              er_grid[:].rearrange("p (g h) -> p g h", g=G))
            deg_sb = cst.tile([P, G], FP)
            nc.sync.dma_start(deg_sb[:], deg[:])

            k0 = 0
            for g in range(len(gds)):
                dd = gds[g]
                idx_t = ip.tile([P, dd], I32, tag="idx")
                nc.sync.dma_start(idx_t[:], idx[:, k0:k0 + dd])
                g_t = gp.tile([P, dd, H], FP, tag="g")
                for k in range(dd):
                    nc.gpsimd.indirect_dma_start(
                        out=g_t[:, k, :],
                        out_offset=None,
                        in_=el8[:],
                        in_offset=bass.IndirectOffsetOnAxis(
                            ap=idx_t[:, k:k + 1], axis=0),
                    )
                # m = g * er ; y = max(m - 1, 0)
                y_t = yp.tile([P, dd, H], FP, tag="y")
                nc.vector.tensor_tensor(
                    y_t[:], g_t[:],
                    er_sb[:, g, :].unsqueeze(1).to_broadcast([P, dd, H]),
                    mybir.AluOpType.mult)
                nc.vector.tensor_scalar(y_t[:], y_t[:], 1.0, 0.0,
                                        mybir.AluOpType.subtract,
                                        mybir.AluOpType.max)
                # s = sum_d y + deg ; r = 1/s
                sums = sp.tile([P, H], FP, tag="sums")
                nc.vector.tensor_reduce(
                    sums[:], y_t[:].rearrange("p d h -> p h d"),
                    mybir.AxisListType.X, mybir.AluOpType.add)
                s_t = sp.tile([P, H], FP, tag="s")
                nc.vector.tensor_tensor(
                    s_t[:], sums[:],
                    deg_sb[:, g:g + 1].to_broadcast([P, H]),
                    mybir.AluOpType.add)
                r_t = sp.tile([P, H], FP, tag="r")
                nc.vector.reciprocal(r_t[:], s_t[:])
                # out = (y + 1) * r   (into the gather tile, then store)
                nc.vector.scalar_tensor_tensor(
                    g_t[:], y_t[:], 1.0,
                    r_t[:].unsqueeze(1).to_broadcast([P, dd, H]),
                    mybir.AluOpType.add, mybir.AluOpType.mult)
                nc.sync.dma_start(
                    out[:, k0 * H:(k0 + dd) * H],
                    g_t[:].rearrange("p k h -> p (k h)"))
                k0 += dd

    nc.compile()
    return nc


# ---------------------------------------------------------------------------
# Host orchestration
# ---------------------------------------------------------------------------

def kernel(feat, etype, src, dst, W_fc, edge_emb, W_e, attn_l, attn_r, attn_e):
    feat = np.asarray(feat)
    etype = np.asarray(etype).astype(np.int64)
    src = np.asarray(src).astype(np.int64)
    dst = np.asarray(dst).astype(np.int64)
    W_fc = np.asarray(W_fc)
    edge_emb = np.asarray(edge_emb)
    W_e = np.asarray(W_e)
    attn_l = np.asarray(attn_l)
    attn_r = np.asarray(attn_r)
    attn_e = np.asarray(attn_e)

    # ---------------- Launch A ----------------
    nc_a = _build_launch_a()
    attn_lr = np.concatenate(
        [attn_l.reshape(1, H * O), attn_r.reshape(1, H * O)], axis=1)
    in_maps_a = []
    for s in range(NCORES):
        featT_s = np.zeros((IN, NSP), np.float32)
        featT_s[:, :NS] = feat[s * NS:(s + 1) * NS].T
        in_maps_a.append({
            "featT": featT_s,
            "w_fc": W_fc.astype(np.float32),
            "attn_lr": np.broadcast_to(attn_lr.astype(np.float32), (P, 2 * H * O)).copy(),
            "edge_embT": np.ascontiguousarray(edge_emb.T.astype(np.float32)),
            "w_e": W_e.astype(np.float32),
            "attn_e": np.broadcast_to(attn_e.reshape(1, H * F).astype(np.float32), (T, H * F)).copy(),
        })
    res_a = run_bass_kernel_spmd(nc_a, in_maps_a, core_ids=list(range(NCORES)))

    # el8 row(n, t) = (12544*(n//12500) + n%12500)*T + t ; tail rows zero pad
    el8_full = np.zeros((EL8_ROWS, H), np.float32)
    er_all = np.zeros((NCORES, NSP, H), np.float32)
    for s in range(NCORES):
        el8_full[s * NSP * T:(s + 1) * NSP * T] = res_a.results[s]["el8s"]
        er_all[s] = res_a.results[s]["erp"]

    # ---------------- host index construction (integers only) -------------
    # edges to cores by dst range; dst-sort within core
    core_of = dst // NS
    order_all = np.argsort(core_of * (2 * N) + dst, kind="stable")

    per_core = []
    for c in range(NCORES):
        lo = np.searchsorted(core_of[order_all], c, side="left")
        hi = np.searchsorted(core_of[order_all], c, side="right")
        per_core.append(order_all[lo:hi])

    # degree-sorted node grouping per core (shared chunk structure)
    node_perm = np.zeros((NCORES, NSP), np.int64)   # grid pos -> local node
    degrees = np.zeros((NCORES, NSP), np.int64)
    for c in range(NCORES):
        e_ids = per_core[c]
        ld = dst[e_ids] - c * NS
        cnt = np.bincount(ld, minlength=NSP)
        perm = np.argsort(cnt, kind="stable")       # ascending degree
        node_perm[c] = perm
        degrees[c] = cnt[perm]

    # groups: grid position (p, g) -> node_perm[g*128 + p]  (sorted order runs
    # down the group-axis first so consecutive groups have similar degrees)
    # group g covers sorted positions [g*128, (g+1)*128)
    gmax = degrees.reshape(NCORES, G, P).max(axis=2).max(axis=0)  # [G]

    # per-group slot width
    gds = [int(max(d, 1)) for d in gmax]
    ktot = sum(gds)

    nc_b = _build_launch_b(gds, ktot)

    # per-core B inputs
    in_maps_b = []
    slot_edge = np.full((NCORES, P, ktot), -1, np.int64)  # slot -> edge id
    for c in range(NCORES):
        e_ids = per_core[c]                      # dst-sorted edge ids
        ld = dst[e_ids] - c * NS
        cnt = np.bincount(ld, minlength=NSP)
        starts = np.concatenate([[0], np.cumsum(cnt)])
        perm = node_perm[c]
        inv_sorted_pos = np.empty(NSP, np.int64)
        inv_sorted_pos[perm] = np.arange(NSP)

        colbase = np.concatenate([[0], np.cumsum(gds)[:-1]]).astype(np.int64)

        nodes_pg = perm.reshape(G, P)                    # grid (g, p) -> node
        er_grid = er_all[c][nodes_pg].transpose(1, 0, 2)  # [P, G, H]
        deg_np = np.maximum(cnt[nodes_pg], 1).T.astype(np.float32)  # [P, G]

        # vectorized per-edge slot assignment (e_ids is dst-sorted)
        ld = dst[e_ids] - c * NS
        rank = np.arange(len(e_ids)) - starts[ld]
        spos = inv_sorted_pos[ld]
        gg_ = spos // P
        pp_ = spos % P
        cols = colbase[gg_] + rank
        rows = (src[e_ids] // NS) * NSP + (src[e_ids] % NS)
        idx_np = np.full((P, ktot), SENTINEL * T, np.int64)
        idx_np[pp_, cols] = rows * T + etype[e_ids]
        slot_edge[c, pp_, cols] = e_ids

        in_maps_b.append({
            "el8": el8_full,
            "er_grid": er_grid.reshape(P, G * H),
            "deg": deg_np,
            "idx": idx_np.astype(np.int32),
        })

    res_b = run_bass_kernel_spmd(nc_b, in_maps_b, core_ids=list(range(NCORES)))

    # ---------------- unshard ----------------
    out = np.zeros((E, H), np.float32)
    for c in range(NCORES):
        o_c = res_b.results[c]["out"].reshape(P, ktot, H)
        mask = slot_edge[c] >= 0
        out[slot_edge[c][mask]] = o_c[mask]

    # timing estimate via the cost-model simulator (no NTFF profiling
    # available under this axon client; see test.py)
    try:
        from concourse.timeline_sim import TimelineSim
        _timings["A_ns"] = TimelineSim(nc_a).simulate()
        _timings["B_ns"] = TimelineSim(nc_b).simulate()
    except Exception as ex:  # timing must never break correctness
        _timings["error"] = repr(ex)

    return out



# revision 5
# speedup vs baseline: 2.7772x; 2.7772x over previous
"""Trainium2 Bass kernel for nn_AttentionWeight (GAT edge softmax), v2.

out[e,h] = softmax_over_dst_segments(relu(el[src]+er[dst]+ee[etype]))

Math on device (same as v1):
  exp(relu(x)) = max(exp(x), 1),  exp(x) = exp(el)*exp(ee)*exp(er)
  y := max(m - 1, 0) with m = exp(el)*exp(ee)*exp(er); padding slots give m=0
  s = sum(y) + deg ;  out = (y + 1) * (1/s)

Distribution (8 NeuronCores):
  Launch A: node-sharded projections -> elp=exp(el), erp=exp(er) [NSP,8] f32
    and eeb=exp(ee) [T,H] bf16.
  Host (integer indexing only): assembles a 256B-strided node table
    tbl[row(n), 0:8] = elp[n] (6 zero sentinel rows interleaved), packs edges
    into a dst-grid whose columns are split into 6 overlapping 32768-row
    index windows (int16 limit of InstDMAGatherAnt), 2-choice load-balanced.
  Launch B: dst-sharded edge softmax. Gathers use raw InstDMAGatherAnt
    (32B payload per index, 256B row stride, <=1024 idxs/instruction --
    hardware descriptor-ring cap probed empirically). ee applied via a
    host-permuted bf16 stream; er via per-(group) broadcast; segment sums
    via per-section strided reduces into a partials tile.
  Host: scatters grid slots back to edge order.

All FP arithmetic on device; host only shards/permutes/concatenates and
builds integer index arrays.
"""

import sys

sys.path.insert(0, "/opt/trn_rl_repo")

import numpy as np

import concourse.bass as bass
import concourse.bacc as bacc
import concourse.mybir as mybir
import concourse.tile as tile
from concourse.library_config import mlp as mlp_lib
from concourse.bass_utils import run_bass_kernel_spmd

N = 100000
E = 3200000
IN = 256
H = 8
O = 64
F = 64
T = 8
NCORES = 8
P = 128

NS = N // NCORES            # 12500 nodes per shard
NSP = 12544                 # padded to 128*98
G = NSP // P                # 98 groups of 128 dst nodes
NB = 7                      # group-blocks
GB = G // NB                # 14 groups per block

WSTRIDE = 16896             # window stride; window w covers rows [s*w, s*w+32768)
NWIN = 6
WLEN = 32768
SENT_OFF = 8448             # window-local index of the zero sentinel row
SENT_POS = WSTRIDE * np.arange(NWIN) + SENT_OFF   # table rows that are zero
R_TBL = N + 480 + NWIN      # 100480 node rows (padded) + 6 sentinels
NMAX_I = 1024               # max idxs per gather instruction (HW ring cap)

FP = mybir.dt.float32
BF = mybir.dt.bfloat16
I16 = mybir.dt.int16

_timings = {}


def _row_of_node(n):
    """Table row of node n after interleaving the zero sentinel rows."""
    row = np.asarray(n, np.int64).copy()
    for s in np.sort(SENT_POS):
        row += row >= s
    return row


# ---------------------------------------------------------------------------
# Launch A: projections
# ---------------------------------------------------------------------------

def _build_launch_a():
    nc = bacc.Bacc("TRN2", target_bir_lowering=False, debug=False,
                   num_devices=NCORES)
    featT = nc.dram_tensor("featT", [IN, NSP], FP, kind="ExternalInput")
    w_fc = nc.dram_tensor("w_fc", [IN, H * O], FP, kind="ExternalInput")
    attn_lr = nc.dram_tensor("attn_lr", [P, 2 * H * O], FP, kind="ExternalInput")
    edge_embT = nc.dram_tensor("edge_embT", [F, T], FP, kind="ExternalInput")
    w_e = nc.dram_tensor("w_e", [F, H * F], FP, kind="ExternalInput")
    attn_e = nc.dram_tensor("attn_e", [T, H * F], FP, kind="ExternalInput")
    elp = nc.dram_tensor("elp", [NSP, H], FP, kind="ExternalOutput")
    erp = nc.dram_tensor("erp", [NSP, H], FP, kind="ExternalOutput")
    eeb = nc.dram_tensor("eeb", [T, H], BF, kind="ExternalOutput")

    with tile.TileContext(nc) as tc:
        with (
            tc.tile_pool(name="sb", bufs=1) as sb,
            tc.tile_pool(name="mm", bufs=2) as mm,
            tc.tile_pool(name="ps", bufs=2, space="PSUM") as ps,
        ):
            # wl/wr: contract W_fc[i, h*O+o] with attn_l/r[h, o] -> [i, 2H]
            wfc_t = [sb.tile([P, H * O], FP, tag=f"wfc{c}", name=f"wfc{c}")
                     for c in range(2)]
            for c in range(2):
                nc.sync.dma_start(wfc_t[c][:], w_fc[c * P:(c + 1) * P, :])
            alr_t = sb.tile([P, 2 * H * O], FP)
            nc.sync.dma_start(alr_t[:], attn_lr[:])
            wlr = [sb.tile([P, 2 * H], FP, tag=f"wlr{c}", name=f"wlr{c}")
                   for c in range(2)]
            for c in range(2):
                for half in range(2):
                    tmp = mm.tile([P, H * O], FP, tag="wtmp")
                    nc.vector.tensor_tensor(
                        tmp[:], wfc_t[c][:],
                        alr_t[:, half * H * O:(half + 1) * H * O],
                        mybir.AluOpType.mult)
                    nc.vector.tensor_reduce(
                        wlr[c][:, half * H:(half + 1) * H],
                        tmp[:].rearrange("p (h o) -> p h o", h=H),
                        mybir.AxisListType.X, mybir.AluOpType.add)

            # ee table: (edge_emb @ W_e) [T, H*F] contract attn_e -> exp -> bf16
            embT_t = sb.tile([F, T], FP)
            nc.sync.dma_start(embT_t[:], edge_embT[:])
            we_t = sb.tile([F, H * F], FP)
            nc.sync.dma_start(we_t[:], w_e[:])
            ae_t = sb.tile([T, H * F], FP)
            nc.sync.dma_start(ae_t[:], attn_e[:])
            proj_ps = ps.tile([T, H * F], FP)
            nc.tensor.matmul(proj_ps[:], lhsT=embT_t[:], rhs=we_t[:],
                             start=True, stop=True)
            proj_sb = sb.tile([T, H * F], FP)
            nc.vector.tensor_tensor(
                proj_sb[:], proj_ps[:], ae_t[:], mybir.AluOpType.mult)
            ee_sb = sb.tile([T, H], FP)
            nc.vector.tensor_reduce(
                ee_sb[:], proj_sb[:].rearrange("t (h f) -> t h f", h=H),
                mybir.AxisListType.X, mybir.AluOpType.add)
            eep_sb = sb.tile([T, H], BF)
            nc.scalar.activation(eep_sb[:], ee_sb[:],
                                 mybir.ActivationFunctionType.Exp)
            nc.sync.dma_start(eeb[:], eep_sb[:])

            # el/er for the shard; node ln = p*G + tt
            ftT = [sb.tile([P, NSP], FP, tag=f"ft{c}", name=f"ft{c}")
                   for c in range(2)]
            for c in range(2):
                nc.sync.dma_start(ftT[c][:], featT[c * P:(c + 1) * P, :])
            elr = sb.tile([P, G, 2 * H], FP)
            SLICES = 32
            tt = 0
            while tt < G:
                nsl = min(SLICES, G - tt)
                bank = ps.tile([P, SLICES * 2 * H], FP, tag="bank")
                for j in range(nsl):
                    sl = bank[:, j * 2 * H:(j + 1) * 2 * H]
                    for c in range(2):
                        lhsT = ftT[c][:].rearrange(
                            "i (p t) -> i t p", p=P)[:, tt + j, :]
                        nc.tensor.matmul(sl, lhsT=lhsT, rhs=wlr[c][:],
                                         start=(c == 0), stop=(c == 1))
                nc.scalar.activation(
                    elr[:, tt:tt + nsl, :],
                    bank[:, :nsl * 2 * H].rearrange("p (t h) -> p t h",
                                                    h=2 * H),
                    mybir.ActivationFunctionType.Exp)
                tt += nsl
            nc.sync.dma_start(
                elp[:].rearrange("(p t) h -> p t h", p=P), elr[:, :, 0:H])
            nc.sync.dma_start(
                erp[:].rearrange("(p t) h -> p t h", p=P), elr[:, :, H:2 * H])

    nc.compile()
    return nc


# ---------------------------------------------------------------------------
# Launch B: windowed gather + edge softmax
# ---------------------------------------------------------------------------

def _raw_dma_gather(nc, out_ap, in_ap, idxs_ap, num_idxs, elem_size,
                    elem_step):
    g = nc.gpsimd
    assert in_ap.ap[0][0] == elem_step
    stride_bytes = elem_step * mybir.dt.size(in_ap.dtype)
    assert stride_bytes % 256 == 0
    _in_ap = g.lower_ap_dma(in_ap, for_custom_bir_dma=True)
    _idxs_ap = g.lower_ap(idxs_ap)
    _out_ap = g.lower_ap(out_ap)
    return g.add_instruction(
        mybir.InstDMAGatherAnt(
            name=nc.get_next_instruction_name(),
            ins=[*_in_ap, _idxs_ap, g.lower_val_access(g.to_reg(num_idxs))],
            outs=[_out_ap],
            transpose=False, num_idxs=num_idxs, elem_size=elem_size,
            stride_bytes_256=stride_bytes // 256, gen_mode=0,
            single_packet=True, queue_num=0, sbuf_tokens_per_rank=0,
            sbuf_free_dim_per_rank=0, sbuf_free_dim_pad_per_rank=0,
            sbuf_byte_offset=0,
        ))


def _build_launch_b(Cgw, ktot2):
    """Cgw[g, w]: column width of section (group g, window w); shared by all
    cores. Columns are laid out block-major: for each block B (14 groups),
    for each window w, for each g in B: C[g, w] columns."""
    nc = bacc.Bacc("TRN2", target_bir_lowering=False, debug=False,
                   num_devices=NCORES)
    tbl = nc.dram_tensor("tbl", [R_TBL, 64], FP, kind="ExternalInput")
    idx_h = nc.dram_tensor("idx_h", [P, ktot2 * 8], I16, kind="ExternalInput")
    ee_h = nc.dram_tensor("ee_h", [P, ktot2 * H], BF, kind="ExternalInput")
    er_h = nc.dram_tensor("er_h", [P, G * H], FP, kind="ExternalInput")
    deg_h = nc.dram_tensor("deg_h", [P, G], FP, kind="ExternalInput")
    out_h = nc.dram_tensor("out_h", [P, ktot2 * H], FP, kind="ExternalOutput")

    with tile.TileContext(nc) as tc:
        with (
            tc.tile_pool(name="cst", bufs=1) as cst,
            tc.tile_pool(name="gp", bufs=3) as gp,
            tc.tile_pool(name="ep", bufs=3) as ep,
            tc.tile_pool(name="ip", bufs=3) as ip,
            tc.tile_pool(name="sp", bufs=3) as sp,
        ):
            nc.gpsimd.load_library(mlp_lib)
            er_sb = cst.tile([P, G, H], FP)
            nc.sync.dma_start(er_sb[:],
                              er_h[:].rearrange("p (g h) -> p g h", g=G))
            deg_sb = cst.tile([P, G], FP)
            nc.sync.dma_start(deg_sb[:], deg_h[:])

            col0 = 0
            for b in range(NB):
                gs = list(range(b * GB, (b + 1) * GB))
                bcols = int(sum(Cgw[g, w] for g in gs for w in range(NWIN)))
                if bcols == 0:
                    continue
                idx_t = ip.tile([P, bcols * 8], I16, tag="idx")
                nc.sync.dma_start(idx_t[:],
                                  idx_h[:, col0 * 8:(col0 + bcols) * 8])
                ee_t = ep.tile([P, bcols, H], BF, tag="ee")
                nc.sync.dma_start(
                    ee_t[:],
                    ee_h[:, col0 * H:(col0 + bcols) * H]
                    .rearrange("p (c h) -> p c h", h=H))
                g_t = gp.tile([P, bcols, H], FP, tag="g")

                # gathers: per window, split into <=8-column instructions
                lc = 0
                for w in range(NWIN):
                    wcols = int(sum(Cgw[g, w] for g in gs))
                    base = WSTRIDE * w
                    c = 0
                    while c < wcols:
                        cc = min(8, wcols - c)
                        _raw_dma_gather(
                            nc, g_t[:, lc + c:lc + c + cc, :],
                            tbl[base:, 0:8],
                            idx_t[:, (lc + c) * 8:(lc + c + cc) * 8],
                            cc * P, 8, 64)
                        c += cc
                    lc += wcols

                # m = g * ee
                nc.vector.tensor_tensor(g_t[:], g_t[:], ee_t[:],
                                        mybir.AluOpType.mult)
                # m *= er (per section broadcast)
                lc = 0
                for w in range(NWIN):
                    for g in gs:
                        cw = int(Cgw[g, w])
                        if cw:
                            nc.vector.tensor_tensor(
                                g_t[:, lc:lc + cw, :], g_t[:, lc:lc + cw, :],
                                er_sb[:, g, :].unsqueeze(1)
                                .to_broadcast([P, cw, H]),
                                mybir.AluOpType.mult)
                        lc += cw
                # y = max(m - 1, 0)
                nc.vector.tensor_scalar(g_t[:], g_t[:], 1.0, 0.0,
                                        mybir.AluOpType.subtract,
                                        mybir.AluOpType.max)
                # partial sums per section (every section non-empty, so the
                # partials tile is fully written each block)
                part = sp.tile([P, GB, NWIN, H], FP, tag="part")
                lc = 0
                for w in range(NWIN):
                    for j, g in enumerate(gs):
                        cw = int(Cgw[g, w])
                        if cw:
                            nc.vector.tensor_reduce(
                                part[:, j, w, :],
                                g_t[:, lc:lc + cw, :]
                                .rearrange("p c h -> p h c"),
                                mybir.AxisListType.X, mybir.AluOpType.add)
                        lc += cw
                # s = sum_w partials + deg ; r = 1/s
                sums = sp.tile([P, GB, H], FP, tag="sums")
                nc.vector.tensor_reduce(
                    sums[:], part[:].rearrange("p j w h -> p j h w"),
                    mybir.AxisListType.X, mybir.AluOpType.add)
                s_t = sp.tile([P, GB, H], FP, tag="s")
                nc.vector.tensor_tensor(
                    s_t[:], sums[:],
                    deg_sb[:, b * GB:(b + 1) * GB].unsqueeze(2)
                    .to_broadcast([P, GB, H]),
                    mybir.AluOpType.add)
                r_t = sp.tile([P, GB, H], FP, tag="r")
                nc.vector.reciprocal(r_t[:], s_t[:])
                # out = (y + 1) * r (per section broadcast)
                lc = 0
                for w in range(NWIN):
                    for j, g in enumerate(gs):
                        cw = int(Cgw[g, w])
                        if cw:
                            nc.vector.scalar_tensor_tensor(
                                g_t[:, lc:lc + cw, :], g_t[:, lc:lc + cw, :],
                                1.0,
                                r_t[:, j, :].unsqueeze(1)
                                .to_broadcast([P, cw, H]),
                                mybir.AluOpType.add, mybir.AluOpType.mult)
                        lc += cw
                nc.sync.dma_start(
                    out_h[:, col0 * H:(col0 + bcols) * H],
                    g_t[:].rearrange("p c h -> p (c h)"))
                col0 += bcols

    nc.compile()
    return nc


# ---------------------------------------------------------------------------
# Host orchestration
# ---------------------------------------------------------------------------

def _rank_within(keys):
    """Rank of each element within its key group (keys need not be sorted)."""
    order = np.argsort(keys, kind="stable")
    ks = keys[order]
    starts = np.empty(len(ks), np.int64)
    new = np.ones(len(ks), bool)
    new[1:] = ks[1:] != ks[:-1]
    idx_of_start = np.flatnonzero(new)
    grp = np.cumsum(new) - 1
    starts = idx_of_start[grp]
    rank_sorted = np.arange(len(ks)) - starts
    rank = np.empty(len(ks), np.int64)
    rank[order] = rank_sorted
    return rank


def kernel(feat, etype, src, dst, W_fc, edge_emb, W_e, attn_l, attn_r, attn_e):
    feat = np.asarray(feat)
    etype = np.asarray(etype).astype(np.int64)
    src = np.asarray(src).astype(np.int64)
    dst = np.asarray(dst).astype(np.int64)
    W_fc = np.asarray(W_fc)
    edge_emb = np.asarray(edge_emb)
    W_e = np.asarray(W_e)
    attn_l = np.asarray(attn_l)
    attn_r = np.asarray(attn_r)
    attn_e = np.asarray(attn_e)

    # ---------------- Launch A ----------------
    nc_a = _build_launch_a()
    attn_lr = np.concatenate(
        [attn_l.reshape(1, H * O), attn_r.reshape(1, H * O)], axis=1)
    in_maps_a = []
    for s in range(NCORES):
        featT_s = np.zeros((IN, NSP), np.float32)
        featT_s[:, :NS] = feat[s * NS:(s + 1) * NS].T
        in_maps_a.append({
            "featT": featT_s,
            "w_fc": W_fc.astype(np.float32),
            "attn_lr": np.broadcast_to(
                attn_lr.astype(np.float32), (P, 2 * H * O)).copy(),
            "edge_embT": np.ascontiguousarray(edge_emb.T.astype(np.float32)),
            "w_e": W_e.astype(np.float32),
            "attn_e": np.broadcast_to(
                attn_e.reshape(1, H * F).astype(np.float32), (T, H * F)).copy(),
        })
    res_a = run_bass_kernel_spmd(nc_a, in_maps_a, core_ids=list(range(NCORES)))

    # node table [R_TBL, 64] f32 (integer assembly of device outputs)
    el_full = np.concatenate(
        [np.asarray(res_a.results[s]["elp"])[:NS] for s in range(NCORES)], axis=0)
    er_all = np.stack(
        [np.asarray(res_a.results[s]["erp"]) for s in range(NCORES)], axis=0)
    ee_bf = np.asarray(res_a.results[0]["eeb"])            # [T, H] bf16
    tbl_np = np.zeros((R_TBL, 64), np.float32)
    rows_nodes = _row_of_node(np.arange(N))
    tbl_np[rows_nodes, :H] = el_full

    # ---------------- host index construction (integers only) -------------
    core_of = dst // NS
    order_all = np.argsort(core_of * (2 * N) + dst, kind="stable")
    per_core = []
    for c in range(NCORES):
        lo = np.searchsorted(core_of[order_all], c, side="left")
        hi = np.searchsorted(core_of[order_all], c, side="right")
        per_core.append(order_all[lo:hi])

    # degree-sorted node grouping per core (shared group structure)
    node_perm = np.zeros((NCORES, NSP), np.int64)
    for c in range(NCORES):
        ld = dst[per_core[c]] - c * NS
        cnt = np.bincount(ld, minlength=NSP)
        node_perm[c] = np.argsort(cnt, kind="stable")

    # window assignment per edge (2-choice balancing), per core
    srow = _row_of_node(src)                       # table row of each src
    lo_w = np.maximum(0, -(-(srow - (WLEN - 1)) // WSTRIDE))
    hi_w = np.minimum(NWIN - 1, srow // WSTRIDE)
    loads_all = np.zeros((NCORES, NSP, NWIN), np.int64)
    w_edge = np.zeros(E, np.int64)
    for c in range(NCORES):
        e_ids = per_core[c]
        ld = dst[e_ids] - c * NS
        lw, hw = lo_w[e_ids], hi_w[e_ids]
        loads = loads_all[c]
        forced = lw == hw
        np.add.at(loads, (ld[forced], lw[forced]), 1)
        o = np.zeros((NSP, NWIN), np.int64)
        flex = ~forced
        np.add.at(o, (ld[flex], hw[flex]), 1)
        x = np.zeros((NSP, NWIN), np.int64)
        for z in range(1, NWIN):
            xz = np.clip((o[:, z] + loads[:, z] - loads[:, z - 1]) // 2,
                         0, o[:, z])
            x[:, z] = xz
            loads[:, z - 1] += xz
            loads[:, z] += o[:, z] - xz
        # per-edge: flexible edges ranked within (row, zone)
        we = lw.copy()
        fidx = np.flatnonzero(flex)
        if len(fidx):
            key = ld[fidx] * NWIN + hw[fidx]
            rk = _rank_within(key)
            go_low = rk < x[ld[fidx], hw[fidx]]
            we[fidx] = np.where(go_low, hw[fidx] - 1, hw[fidx])
        w_edge[e_ids] = we

    # shared section widths C[g, w] = max over cores and rows in group
    Cgw = np.zeros((G, NWIN), np.int64)
    for c in range(NCORES):
        pg = node_perm[c].reshape(G, P)            # grid (g, p) -> node
        Cgw = np.maximum(Cgw, loads_all[c][pg].max(axis=1))
    Cgw = np.maximum(Cgw, 1)       # no empty sections (partials stay valid)
    ktot2 = int(Cgw.sum())

    # block-major column offsets: for b, for w, for g in block
    secbase = np.zeros((G, NWIN), np.int64)
    col = 0
    for b in range(NB):
        for w in range(NWIN):
            for g in range(b * GB, (b + 1) * GB):
                secbase[g, w] = col
                col += Cgw[g, w]
    assert col == ktot2

    nc_b = _build_launch_b(Cgw, ktot2)

    in_maps_b = []
    slot_edge = np.full((NCORES, P, ktot2), -1, np.int64)
    for c in range(NCORES):
        e_ids = per_core[c]
        ld = dst[e_ids] - c * NS
        perm = node_perm[c]
        inv_pos = np.empty(NSP, np.int64)
        inv_pos[perm] = np.arange(NSP)
        spos = inv_pos[ld]
        gg = spos // P
        pp = spos % P
        we = w_edge[e_ids]
        rk = _rank_within(ld * NWIN + we)          # rank within (node, window)
        cols = secbase[gg, we] + rk
        assert (rk < Cgw[gg, we]).all()

        # idx values: window-local table row of src
        off = srow[e_ids] - we * WSTRIDE
        assert (off >= 0).all() and (off < WLEN).all()
        idx_np = np.full((P, ktot2), SENT_OFF, np.int16)
        idx_np[pp, cols] = off.astype(np.int16)
        slot_edge[c, pp, cols] = e_ids

        ee_np = np.zeros((P, ktot2, H), ee_bf.dtype)
        ee_np[pp, cols] = ee_bf[etype[e_ids]]

        # wrap idx into the 16-partition index layout.  Every gather
        # instruction starts at a grid-column boundary, which makes the
        # wrapped layout boundary-independent: idx_h[q, m] holds the index
        # of grid position (p=(m%8)*16 + q%16, col=m//8).
        idx_lin = idx_np.T.reshape(-1)             # pos = col*128 + p
        idx_wrapped = np.zeros((P, ktot2 * 8), np.int16)
        lin = idx_lin.reshape(-1, 16)              # [ktot2*8, 16]
        for r16 in range(16):
            idx_wrapped[r16::16, :] = lin[:, r16]

        nodes_pg = perm.reshape(G, P)
        er_grid = er_all[c][nodes_pg].transpose(1, 0, 2)   # [P, G, H]
        cnt = np.bincount(ld, minlength=NSP)
        deg_np = np.maximum(cnt[nodes_pg], 1).T.astype(np.float32)

        in_maps_b.append({
            "tbl": tbl_np,
            "idx_h": idx_wrapped,
            "ee_h": ee_np.reshape(P, ktot2 * H),
            "er_h": er_grid.reshape(P, G * H).astype(np.float32),
            "deg_h": deg_np,
        })

    res_b = run_bass_kernel_spmd(nc_b, in_maps_b, core_ids=list(range(NCORES)))

    out = np.zeros((E, H), np.float32)
    for c in range(NCORES):
        o_c = np.asarray(res_b.results[c]["out_h"]).reshape(P, ktot2, H)
        mask = slot_edge[c] >= 0
        out[slot_edge[c][mask]] = o_c[mask]

    try:
        from concourse.timeline_sim import TimelineSim
        _timings["A_ns"] = TimelineSim(nc_a).simulate()
        _timings["B_ns"] = TimelineSim(nc_b).simulate()
        _timings["ktot2"] = ktot2
    except Exception as ex:
        _timings["error"] = repr(ex)

    return out
